# revision 2
# baseline (speedup 1.0000x reference)
"""GRU cell (B=4096, H=2048) on 8 TRN2 NeuronCores — fp8/bf16 mixed.

Sharding: data-parallel over batch — each core computes 512 rows; weights
replicated, no collectives.

Per-core compute in transposed space (hidden on partitions, batch free).
All weights are pre-scaled on the host so max|w8| = 0.9995: the weights
are uniform(+-stdv), and placing the max just under a binade boundary
fills e4m3's finest relative grid (quantization MSE x0.73 vs the x64
scale that landed max|w8| at 1.414, mid-binade).  Every activation
descales with scale=1/SW.  The r/z gates and the n-gate's hh half run
entirely as fp8-e4m3 DoubleRow matmuls (2 k-chunks per instruction).
The n gate's ih half keeps NBF=4 bf16 k-chunks + NF8=12 fp8-DR chunks
accumulating into one PSUM bank (shared SW scale makes that legal);
blocks 0-1 run fully fp8 so the bf16 acts stay out of the HBM-bound
startup window.  The hx used in the output blend is fp32 (error margin
at DMA-only cost).  Per block, bf16 and fp8-DR sweeps are grouped and
block handedness alternates so there is ~one FWL<->DoubleRow mode switch
per block.  Startup: a tiny memset warm tile feeds FD=128 fp8-DR warm-up
matmuls that ramp the PE HAM clock while the need-ordered DMA stream on
the sync ring lands block 0's operands.  The last block computes
out = c*n + a with c = sigmoid(-u) = 1-z and a = z*hx prepared during
the matmuls, so only the quartered x/tanh/mul/add chain trails the
final matmul.

Emulated rel err 1.9344e-2 (gate 2e-2); the numpy emulation of the
quantization error predicts the HW rel err to ~4 digits.
"""

from contextlib import ExitStack

import ml_dtypes
import numpy as np

import concourse.bass as bass
import concourse.tile as tile
from concourse import bacc, mybir
from concourse.bass_utils import run_bass_kernel_spmd

H = 2048
B = 4096
N_CORES = 8
BL = B // N_CORES  # 512 batch rows per core
P = 128
NKB = H // P  # 16 contraction chunks
NNB = H // P  # 16 hidden (output) blocks
F32 = mybir.dt.float32
F8 = mybir.dt.float8e4
BF16 = mybir.dt.bfloat16
DR = mybir.MatmulPerfMode.DoubleRow
NBF = 4  # n-gate ih-half k-chunks in bf16 (rest fp8-DR)
NF8 = NKB - NBF  # n-gate ih-half k-chunks in fp8-DR
SWMAX = 0.9995  # target max|w8| — just under the binade boundary

# w8 matrix order: 0 r_ih, 1 r_hh, 2 z_ih, 3 z_hh
# w16/w8n matrix order splits the n-gate ih half: first NBF k-chunks
# bf16, last NF8 fp8-DR.  w8nh: n-gate hh half, fully fp8 (its error is
# attenuated by r inside tanh(gi2 + r*gh2)).  w8nf: full-K fp8 n-ih for
# blocks 0-1.  ALL weights are pre-scaled xSW on the host; every PSUM
# preactivation is SW-scaled and the activations descale with 1/SW.
# b_hh2 is pre-scaled xSW so it can add to the PSUM before the tanh
# descale.


def _build_program(sw: float) -> bacc.Bacc:
    nc = bacc.Bacc(
        "TRN2", target_bir_lowering=False, debug=False, num_devices=N_CORES
    )

    xt8 = nc.dram_tensor("xt8", [P, NKB, BL], F8, kind="ExternalInput").ap()
    hxt8 = nc.dram_tensor("hxt8", [P, NKB, BL], F8, kind="ExternalInput").ap()
    xtb = nc.dram_tensor("xtb", [P, NBF, BL], BF16, kind="ExternalInput").ap()
    hxt32 = nc.dram_tensor("hxt32", [P, NKB, BL], F32, kind="ExternalInput").ap()
    w8 = nc.dram_tensor("w8", [4, NNB, P, NKB, P], F8, kind="ExternalInput").ap()
    w16 = nc.dram_tensor("w16", [NNB, P, NBF, P], BF16, kind="ExternalInput").ap()
    w8n = nc.dram_tensor("w8n", [NNB, P, NF8, P], F8, kind="ExternalInput").ap()
    w8nh = nc.dram_tensor("w8nh", [NNB, P, NKB, P], F8, kind="ExternalInput").ap()
    w8nf = nc.dram_tensor("w8nf", [2, P, NKB, P], F8, kind="ExternalInput").ap()
    b = nc.dram_tensor("b", [P, 5 * NNB], F32, kind="ExternalInput").ap()
    out = nc.dram_tensor("out", [H, BL], F32, kind="ExternalOutput").ap()

    with tile.TileContext(nc) as tc, ExitStack() as ctx:
        const = ctx.enter_context(tc.tile_pool(name="const", bufs=1))
        acts = ctx.enter_context(tc.tile_pool(name="acts", bufs=1))
        wp8 = ctx.enter_context(tc.tile_pool(name="wp8", bufs=18))
        wp16 = ctx.enter_context(tc.tile_pool(name="wp16", bufs=6))
        gates = ctx.enter_context(tc.tile_pool(name="gates", bufs=2))
        opool = ctx.enter_context(tc.tile_pool(name="opool", bufs=3))
        ps_r = ctx.enter_context(tc.tile_pool(name="ps_r", bufs=2, space="PSUM"))
        ps_z = ctx.enter_context(tc.tile_pool(name="ps_z", bufs=2, space="PSUM"))
        ps_gi = ctx.enter_context(tc.tile_pool(name="ps_gi", bufs=2, space="PSUM"))
        ps_gh = ctx.enter_context(tc.tile_pool(name="ps_gh", bufs=2, space="PSUM"))

        # PE warm-up: a tiny memset tile feeds FD=128 fp8-DR matmuls that
        # ramp the HAM clock gate while block 0's DMAs land.  gpsimd's
        # queue is otherwise empty so the memset lands right after the
        # framework barrier.
        warm = const.tile([P, 2, P], F8)
        nc.gpsimd.memset(warm[:], 0.0)
        p_warm = ps_gh.tile([P, BL], F32, tag="p_gh", name="p_warm")

        def warm_mms(n):
            for _ in range(n):
                nc.tensor.matmul(
                    p_warm[:, 0:P], lhsT=warm[:], rhs=warm[:],
                    start=True, stop=True, perf_mode=DR,
                )

        warm_mms(12)

        btile = const.tile([P, 5 * NNB], F32)
        nc.scalar.dma_start(btile[:], b[:])
        xt8_sb = acts.tile([P, NKB, BL], F8)
        hxt8_sb = acts.tile([P, NKB, BL], F8)
        xtb_sb = acts.tile([P, NBF, BL], BF16)
        hxt32_sb = acts.tile([P, NKB, BL], F32)

        def w8_slab(m, nb):
            s = wp8.tile([P, NKB, P], F8, tag="w8slab", name=f"w8_{m}_{nb}")
            nc.sync.dma_start(s[:], w8[m, nb])
            return s

        def w16_slab(nb):
            s = wp16.tile([P, NBF, P], BF16, tag="w16slab", name=f"w16_{nb}")
            nc.sync.dma_start(s[:], w16[nb])
            return s

        def w8n_slab(nb):
            s = wp8.tile([P, NF8, P], F8, tag="w8nslab", name=f"w8n_{nb}")
            nc.sync.dma_start(s[:], w8n[nb])
            return s

        def w8nh_slab(nb):
            s = wp8.tile([P, NKB, P], F8, tag="w8slab", name=f"w8nh_{nb}")
            nc.sync.dma_start(s[:], w8nh[nb])
            return s

        def w8nf_slab(i):
            s = wp8.tile([P, NKB, P], F8, tag="w8slab", name=f"w8nf_{i}")
            nc.sync.dma_start(s[:], w8nf[i])
            return s

        def qdma(sb, dram, qi):
            nc.sync.dma_start(
                sb[:, 4 * qi : 4 * qi + 4, :], dram[:, 4 * qi : 4 * qi + 4, :]
            )

        def hx32dma(c0, c1):
            nc.sync.dma_start(hxt32_sb[:, c0:c1, :], hxt32[:, c0:c1, :])

        # Serial need-order on the sync ring: startup is HBM-bound, so one
        # ring in consumption order beats parallel rings.  Block 0 MM order
        # is r-ih, z-ih, r-hh, z-hh, gi, gh; DMAs land in that order with
        # act quarters interleaved so each sweep's first matmul starts as
        # soon as its prefix is in.
        qdma(xt8_sb, xt8, 0)
        s8_rih0 = w8_slab(0, 0)
        qdma(xt8_sb, xt8, 1)
        qdma(xt8_sb, xt8, 2)
        s8_zih0 = w8_slab(2, 0)
        qdma(xt8_sb, xt8, 3)
        qdma(hxt8_sb, hxt8, 0)
        s8_rhh0 = w8_slab(1, 0)
        qdma(hxt8_sb, hxt8, 1)
        qdma(hxt8_sb, hxt8, 2)
        s8_zhh0 = w8_slab(3, 0)
        qdma(hxt8_sb, hxt8, 3)
        s8nf0 = [w8nf_slab(0), w8nh_slab(0)]
        # hx32 chunks 0-1 before block 1's weights: block 0's blend reads
        # chunk 0 right after block 0's matmuls.
        hx32dma(0, 2)
        # nb=1 (all-fp8): its MM order is gi, gh, r, z
        s8nf1 = [w8nf_slab(1), w8nh_slab(1)]
        s8_1 = [w8_slab(m, 1) for m in (0, 1, 2, 3)]
        # nb=2 prefetch (even: bf16 gi first) + the bf16 acts
        s16_2 = w16_slab(2)
        nc.sync.dma_start(xtb_sb[:], xtb[:])
        s8_2 = [w8_slab(0, 2), w8_slab(1, 2)]
        s8n_2 = w8n_slab(2)
        s8_2 += [w8_slab(2, 2)]
        s8nh_2 = w8nh_slab(2)
        s8_2 += [w8_slab(3, 2)]
        hx32dma(2, 4)

        def mm_fp8(psum, slab, act_sb, start, stop):
            """8 DoubleRow matmuls sweeping all 16 k-chunks."""
            for j in range(NKB // 2):
                nc.tensor.matmul(
                    psum[:],
                    lhsT=slab[:, 2 * j : 2 * j + 2, :],
                    rhs=act_sb[:, 2 * j : 2 * j + 2, :],
                    start=(start and j == 0),
                    stop=(stop and j == NKB // 2 - 1),
                    perf_mode=DR,
                )

        def mm_n_bf(psum, s16, actb, start=True, stop=False):
            """n-gate ih half, bf16 segment (k-chunks 0..NBF-1)."""
            for k in range(NBF):
                nc.tensor.matmul(
                    psum[:],
                    lhsT=s16[:, k, :],
                    rhs=actb[:, k, :],
                    start=(start and k == 0),
                    stop=(stop and k == NBF - 1),
                )

        def mm_n_f8(psum, s8n, act8, start=False, stop=True):
            """n-gate ih half, fp8-DR segment (k-chunks NBF..15)."""
            for j in range(NF8 // 2):
                nc.tensor.matmul(
                    psum[:],
                    lhsT=s8n[:, 2 * j : 2 * j + 2, :],
                    rhs=act8[:, NBF + 2 * j : NBF + 2 * j + 2, :],
                    start=(start and j == 0),
                    stop=(stop and j == NF8 // 2 - 1),
                    perf_mode=DR,
                )

        for nb in range(NNB):
            if nb == 0:
                s8 = [s8_rih0, s8_rhh0, s8_zih0, s8_zhh0]
                s8nf = s8nf0
            elif nb == 1:
                s8 = s8_1
                s8nf = s8nf1
            elif nb == 2:
                s8 = s8_2
                s16 = s16_2
                s8n = s8n_2
                s8nh = s8nh_2
            else:
                # DMA in consumption order (differs by block parity);
                # hx32 blend chunks ride along pairwise.
                s8 = [None] * 4
                if nb % 2 == 0:
                    s16 = w16_slab(nb)
                    s8[0] = w8_slab(0, nb)
                    s8[1] = w8_slab(1, nb)
                    s8n = w8n_slab(nb)
                    s8[2] = w8_slab(2, nb)
                    s8nh = w8nh_slab(nb)
                    s8[3] = w8_slab(3, nb)
                else:
                    s8n = w8n_slab(nb)
                    s8[0] = w8_slab(0, nb)
                    s8[1] = w8_slab(1, nb)
                    s8nh = w8nh_slab(nb)
                    s8[2] = w8_slab(2, nb)
                    s8[3] = w8_slab(3, nb)
                    s16 = w16_slab(nb)
                if nb <= 9:
                    hx32dma(2 * (nb - 2), 2 * (nb - 1))

            p_r = ps_r.tile([P, BL], F32)
            p_z = ps_z.tile([P, BL], F32)
            p_gi = ps_gi.tile([P, BL], F32)
            p_gh = ps_gh.tile([P, BL], F32)
            if nb == 0:
                mm_fp8(p_r, s8[0], xt8_sb, start=True, stop=False)
                mm_fp8(p_z, s8[2], xt8_sb, start=True, stop=False)
                mm_fp8(p_r, s8[1], hxt8_sb, start=False, stop=True)
                mm_fp8(p_z, s8[3], hxt8_sb, start=False, stop=True)
                mm_fp8(p_gi, s8nf[0], xt8_sb, start=True, stop=True)
                mm_fp8(p_gh, s8nf[1], hxt8_sb, start=True, stop=True)
            elif nb == 1:
                mm_fp8(p_gi, s8nf[0], xt8_sb, start=True, stop=True)
                mm_fp8(p_gh, s8nf[1], hxt8_sb, start=True, stop=True)
                mm_fp8(p_r, s8[0], xt8_sb, start=True, stop=False)
                mm_fp8(p_r, s8[1], hxt8_sb, start=False, stop=True)
                mm_fp8(p_z, s8[2], xt8_sb, start=True, stop=False)
                mm_fp8(p_z, s8[3], hxt8_sb, start=False, stop=True)
            elif nb == NNB - 1:
                # last block: r/z early, gh, then gi segments last so the
                # t quarters overlap the gi sweep and only x/tanh/mul/add
                # trail the final matmul.
                mm_fp8(p_r, s8[0], xt8_sb, start=True, stop=False)
                mm_fp8(p_r, s8[1], hxt8_sb, start=False, stop=True)
                mm_fp8(p_z, s8[2], xt8_sb, start=True, stop=False)
                mm_fp8(p_z, s8[3], hxt8_sb, start=False, stop=True)
                mm_fp8(p_gh, s8nh, hxt8_sb, start=True, stop=True)
                mm_n_bf(p_gi, s16, xtb_sb)
                mm_n_f8(p_gi, s8n, xt8_sb)
            elif nb % 2 == 0:
                # Even steady blocks: bf16 segment first, then all fp8-DR.
                # Odd blocks run mirrored so consecutive blocks join
                # same-mode: ~one FWL<->DR switch per block.
                mm_n_bf(p_gi, s16, xtb_sb)
                mm_fp8(p_r, s8[0], xt8_sb, start=True, stop=False)
                mm_fp8(p_r, s8[1], hxt8_sb, start=False, stop=True)
                mm_n_f8(p_gi, s8n, xt8_sb)
                mm_fp8(p_z, s8[2], xt8_sb, start=True, stop=False)
                mm_fp8(p_gh, s8nh, hxt8_sb, start=True, stop=True)
                mm_fp8(p_z, s8[3], hxt8_sb, start=False, stop=True)
            else:
                # Odd steady blocks: DR sweeps first, bf16 segment closes.
                mm_n_f8(p_gi, s8n, xt8_sb, start=True, stop=False)
                mm_fp8(p_r, s8[0], xt8_sb, start=True, stop=False)
                mm_fp8(p_r, s8[1], hxt8_sb, start=False, stop=True)
                mm_fp8(p_gh, s8nh, hxt8_sb, start=True, stop=True)
                mm_fp8(p_z, s8[2], xt8_sb, start=True, stop=False)
                mm_fp8(p_z, s8[3], hxt8_sb, start=False, stop=True)
                mm_n_bf(p_gi, s16, xtb_sb, start=False, stop=True)

            def bias_ap(g):
                return btile[:, g * NNB + nb : g * NNB + nb + 1]

            if nb == NNB - 1:
                # out = c*n + a with c = sigmoid(-u) = 1-z and a = z*hx,
                # both computed while the n matmuls still run; after the
                # final matmul only t/x/tanh/mul/add trail, in quarters,
                # out-DMAs alternating sync/scalar rings.
                r_sb = gates.tile([P, BL], F32, tag="r")
                nc.scalar.activation(
                    r_sb[:], p_r[:], mybir.ActivationFunctionType.Sigmoid,
                    bias=bias_ap(0), scale=1.0 / sw,
                )
                z_sb = gates.tile([P, BL], F32, tag="z")
                nc.scalar.activation(
                    z_sb[:], p_z[:], mybir.ActivationFunctionType.Sigmoid,
                    bias=bias_ap(1), scale=1.0 / sw,
                )
                c_sb = gates.tile([P, BL], F32, tag="d")
                nc.scalar.activation(
                    c_sb[:], p_z[:], mybir.ActivationFunctionType.Sigmoid,
                    bias=bias_ap(4), scale=-1.0 / sw,
                )
                a_sb = gates.tile([P, BL], F32, tag="e")
                nc.vector.tensor_mul(a_sb[:], z_sb[:], hxt32_sb[:, nb, :])
                t_sb = gates.tile([P, BL], F32, tag="t")
                x_sb = gates.tile([P, BL], F32, tag="x")
                n_sb = gates.tile([P, BL], F32, tag="n")
                e2_sb = gates.tile([P, BL], F32, tag="e2")
                o_sb = opool.tile([P, BL], F32, tag="o")
                QH = BL // 4
                # t quarters depend only on p_gh + r, both ready before the
                # gi sweep (emitted last) finishes — they overlap it.
                for q in range(4):
                    qs = slice(q * QH, (q + 1) * QH)
                    nc.vector.scalar_tensor_tensor(
                        t_sb[:, qs], p_gh[:, qs], bias_ap(3), r_sb[:, qs],
                        op0=mybir.AluOpType.add, op1=mybir.AluOpType.mult,
                    )
                for q in range(4):
                    qs = slice(q * QH, (q + 1) * QH)
                    nc.vector.tensor_add(x_sb[:, qs], t_sb[:, qs], p_gi[:, qs])
                    nc.scalar.activation(
                        n_sb[:, qs], x_sb[:, qs],
                        mybir.ActivationFunctionType.Tanh,
                        bias=bias_ap(2), scale=1.0 / sw,
                    )
                    nc.vector.tensor_mul(e2_sb[:, qs], c_sb[:, qs], n_sb[:, qs])
                    nc.vector.tensor_add(o_sb[:, qs], e2_sb[:, qs], a_sb[:, qs])
                    ring = nc.sync if q % 2 == 0 else nc.scalar
                    ring.dma_start(out[nb * P : (nb + 1) * P, qs], o_sb[:, qs])
                continue

            # r = sigmoid(p_r/SW + b_ih0 + b_hh0)
            r_sb = gates.tile([P, BL], F32, tag="r")
            nc.scalar.activation(
                r_sb[:], p_r[:], mybir.ActivationFunctionType.Sigmoid,
                bias=bias_ap(0), scale=1.0 / sw,
            )
            # tanh chain emitted BEFORE the z sigmoid (program order per
            # engine; keeps scalar free for the last block's tanh).
            t_sb = gates.tile([P, BL], F32, tag="t")
            nc.vector.scalar_tensor_tensor(
                t_sb[:], p_gh[:], bias_ap(3), r_sb[:],
                op0=mybir.AluOpType.add, op1=mybir.AluOpType.mult,
            )
            x_sb = gates.tile([P, BL], F32, tag="x")
            nc.vector.tensor_add(x_sb[:], t_sb[:], p_gi[:])
            n_sb = gates.tile([P, BL], F32, tag="n")
            nc.scalar.activation(
                n_sb[:], x_sb[:], mybir.ActivationFunctionType.Tanh,
                bias=bias_ap(2), scale=1.0 / sw,
            )
            # d = hx - n;  hx from the fp32 act copy
            d_sb = gates.tile([P, BL], F32, tag="d")
            nc.vector.tensor_sub(d_sb[:], hxt32_sb[:, nb, :], n_sb[:])
            # z = sigmoid(p_z/SW + b_ih1 + b_hh1), then out = n + z*d
            z_sb = gates.tile([P, BL], F32, tag="z")
            e_sb = gates.tile([P, BL], F32, tag="e")
            o_sb = opool.tile([P, BL], F32, tag="o")
            nc.scalar.activation(
                z_sb[:], p_z[:], mybir.ActivationFunctionType.Sigmoid,
                bias=bias_ap(1), scale=1.0 / sw,
            )
            nc.vector.tensor_mul(e_sb[:], z_sb[:], d_sb[:])
            nc.vector.tensor_add(o_sb[:], n_sb[:], e_sb[:])
            nc.gpsimd.dma_start(out[nb * P : (nb + 1) * P, :], o_sb[:])

    nc.compile()
    return nc


def _pack_inputs(input, hx, weight_ih, weight_hh, bias_ih, bias_hh, sw):
    """Host-side shard + layout packing. Returns per-core input maps."""
    input = np.ascontiguousarray(np.asarray(input, dtype=np.float32))
    hx = np.ascontiguousarray(np.asarray(hx, dtype=np.float32))
    weight_ih = np.asarray(weight_ih, dtype=np.float32)
    weight_hh = np.asarray(weight_hh, dtype=np.float32)
    bias_ih = np.asarray(bias_ih, dtype=np.float32)
    bias_hh = np.asarray(bias_hh, dtype=np.float32)

    # wpack[m, nb, kp, k, n] = W_m[k*128+kp, nb*128+n]
    def wpack(mats, scale, dt):
        return np.ascontiguousarray(
            np.stack(
                [
                    np.asarray(wm * scale, dtype=dt)
                    .reshape(NKB, P, NNB, P)
                    .transpose(2, 1, 0, 3)
                    for wm in mats
                ]
            )
        )

    w8p = wpack(
        [weight_ih[0], weight_hh[0], weight_ih[1], weight_hh[1]],
        sw, ml_dtypes.float8_e4m3,
    )
    wn_f = wpack([weight_ih[2], weight_hh[2]], sw, np.float32)
    w16p = np.ascontiguousarray(wn_f[0, :, :, :NBF, :].astype(ml_dtypes.bfloat16))
    w8np = np.ascontiguousarray(wn_f[0, :, :, NBF:, :].astype(ml_dtypes.float8_e4m3))
    w8nhp = np.ascontiguousarray(wn_f[1].astype(ml_dtypes.float8_e4m3))
    w8nfp = np.ascontiguousarray(wn_f[0, :2].astype(ml_dtypes.float8_e4m3))

    # bpack[p, g*16+nb] = bias_g[nb*128+p]
    # g order: r_sum, z_sum, ih2, hh2, neg_z_sum.  hh2 is xSW because it
    # adds to the SW-scaled PSUM before the tanh descale; neg_z_sum feeds
    # c = sigmoid(-u) = 1-z on the last block.
    bias_all = np.stack(
        [bias_ih[0] + bias_hh[0], bias_ih[1] + bias_hh[1], bias_ih[2],
         np.float32(sw) * bias_hh[2], -(bias_ih[1] + bias_hh[1])]
    )  # [5, H]
    bpack = np.ascontiguousarray(
        bias_all.reshape(5, NNB, P).transpose(2, 0, 1).reshape(P, 5 * NNB)
    )

    def t_pack(a, dt):
        # [BL, H] -> [P, NKB, BL] with [kp, k, m] = a[m, k*128+kp]
        return np.ascontiguousarray(
            a.T.reshape(NKB, P, BL).transpose(1, 0, 2).astype(dt)
        )

    in_maps = []
    for c in range(N_CORES):
        sl = slice(c * BL, (c + 1) * BL)
        in_maps.append(
            {
                "xt8": t_pack(input[sl], ml_dtypes.float8_e4m3),
                "hxt8": t_pack(hx[sl], ml_dtypes.float8_e4m3),
                "xtb": np.ascontiguousarray(
                    t_pack(input[sl], ml_dtypes.bfloat16)[:, :NBF, :]
                ),
                "hxt32": t_pack(hx[sl], np.float32),
                "w8": w8p,
                "w16": w16p,
                "w8n": w8np,
                "w8nh": w8nhp,
                "w8nf": w8nfp,
                "b": bpack,
            }
        )
    return in_maps


_PROGRAM_CACHE = {}


def kernel(input, hx, weight_ih, weight_hh, bias_ih, bias_hh, _trace=False):
    wmax = float(
        max(
            np.abs(np.asarray(weight_ih, dtype=np.float32)).max(),
            np.abs(np.asarray(weight_hh, dtype=np.float32)).max(),
        )
    )
    sw = SWMAX / wmax if wmax > 0 else 64.0
    key = round(sw, 6)
    if key not in _PROGRAM_CACHE:
        _PROGRAM_CACHE[key] = _build_program(sw)
    nc = _PROGRAM_CACHE[key]
    in_maps = _pack_inputs(input, hx, weight_ih, weight_hh, bias_ih, bias_hh, sw)
    res = run_bass_kernel_spmd(nc, in_maps, list(range(N_CORES)), trace=_trace)
    out = np.empty((B, H), dtype=np.float32)
    for c in range(N_CORES):
        out[c * BL : (c + 1) * BL] = res.results[c]["out"].T
    if _trace:
        kernel.last_exec_time_ns = res.exec_time_ns
    return out
